# revision 1
# baseline (speedup 1.0000x reference)
"""Trainium2 Bass kernel for 2-layer GAT (nn_GAT_71622874628670).

Self-contained: accepts FULL inputs, shards across 8 NeuronCores internally,
returns FULL output [50000, 64] f32.

Strategy (1D node sharding, dst-stationary padded CSR):
  - Nodes are permuted by (kA, kB) degree keys and dealt round-robin to 8
    cores; each core owns a 6272-row slab of a global node table.
  - Table row (256B) = [h bf16 x64 | a_src f32 x8 | a_dst f32 x8].
  - conv edge phase: dma_gather by src index (int16 -> table split in A/B
    halves at row 32768; per-tile A/B slot rectangles). Gather layout puts
    slot j of 128 destinations on the 128 partitions, so the segment
    softmax/aggregation = per-partition scalar ops + PSUM-accumulated
    identity matmuls over slot chunks.
  - softmax division deferred to dst level (exp without max-subtraction:
    logits bounded by construction).
  - Cross-core: AllGather of conv1 table slabs; AllGather of transposed
    hidden features for the (redundant per-core) conv2 table build.
"""
import sys
sys.path.insert(0, '/opt/trn_rl_repo')

import numpy as np
import ml_dtypes

import concourse.bass as bass
import concourse.bacc as bacc
import concourse.tile as tile
import concourse.mybir as mybir
from concourse.bass_utils import run_bass_kernel_spmd
from concourse.library_config import mlp
from concourse.masks import make_identity

# ---------------- problem constants (hardcoded; must match reference) ------
N = 50000
E = 1_600_000
C_IN = 512
H1, F1 = 8, 8           # conv1 heads x feat
C_OUT = 64
NEG_SLOPE = 0.2
ELU_ALPHA = 0.2

NCORES = 8
NT = 49                  # dst tiles per core
SH = NT * 128            # 6272 rows per core slab
R = NCORES * SH          # 50176 table rows
ABOUND = 32768           # int16 gather index bound (table A/B split)
NEG_BIG = -10000.0       # dummy-row a_src sentinel -> exp() == 0
CMAX = 160               # max gathered slots per super-tile (SBUF budget)

BF16 = mybir.dt.bfloat16
F32 = mybir.dt.float32
I16 = mybir.dt.int16

TRACE = False            # set True by test harness for NTFF profiling
TRACE_KW = {}
DEBUG_DUMPS = False      # dump intermediates as extra outputs
REPEAT = 1               # repeat body (for delta wall-clock timing)
TIME_RUNS = 0            # extra timed runs of the same NEFF
PHASES = "ABCDEF"        # phases executed on reps >= 1 (ablation timing)
SUBPH = "VMP"            # edge-phase sub-parts: V=dve/act, M=matmuls, P=post


# ================================ HOST PREP ================================

def _host_prep(x, edge_index, W1, as1, ad1, b1, W2, as2, ad2, b2):
    src = edge_index[0].astype(np.int64)
    dst = edge_index[1].astype(np.int64)
    loop = np.arange(N, dtype=np.int64)
    src_all = np.concatenate([src, loop])
    dst_all = np.concatenate([dst, loop])

    deg = np.bincount(dst_all, minlength=N)

    # deal to cores by degree rank; within core sort by degree -> tiles have
    # near-uniform degree
    order = np.argsort(deg, kind="stable")
    rank = np.empty(N, np.int64)
    rank[order] = np.arange(N)
    core = rank % NCORES
    l = rank // NCORES
    key = np.lexsort((deg, core))
    l2 = np.empty(N, np.int64)
    l2[key] = _within_group_index(core[key], NCORES)
    l = l2
    pg = core * SH + l

    # A table = rows [0, ABOUND); B table = rows [OVLO, R). Sources in the
    # overlap [OVLO, ABOUND) can be assigned to either rectangle, absorbing
    # per-node count spread within a tile.
    OVLO = R - ABOUND
    pg_src_n = pg[src_all]
    cat = np.where(pg_src_n < OVLO, 0,
                   np.where(pg_src_n >= ABOUND, 2, 1)).astype(np.int64)
    mA = np.bincount(dst_all[cat == 0], minlength=N)
    mB = np.bincount(dst_all[cat == 2], minlength=N)
    fx = deg - mA - mB

    # per global tile row choose split alpha minimizing D_A + D_B
    t_of_node = l // 128
    best_cost = np.full(NT, 1 << 30)
    best_a = np.zeros(N, np.int64)
    for alpha in range(0, int(deg.max()) + 2):
        aa = np.clip(alpha, mA, mA + fx)
        DA = np.zeros(NT, np.int64)
        DB = np.zeros(NT, np.int64)
        np.maximum.at(DA, t_of_node, aa)
        np.maximum.at(DB, t_of_node, deg - aa)
        cost = DA + DB
        better = cost < best_cost
        if better.any():
            upd = better[t_of_node]
            best_a[upd] = aa[upd]
            best_cost = np.where(better, cost, best_cost)
    a_n = best_a                      # per-node A-slot count
    D_A = np.zeros(NT, np.int64)
    D_B = np.zeros(NT, np.int64)
    np.maximum.at(D_A, t_of_node, a_n)
    np.maximum.at(D_B, t_of_node, deg - a_n)
    D_A = np.maximum(D_A, 1)   # keep >=1 so every tile has a rect
    D_B = np.maximum(D_B, 1)

    # super-tile grouping under column budget
    supers = []       # list of (tile_list, sumA, sumB, rowstart)
    cur, ca, cb = [], 0, 0
    rowstart = 0
    for t in range(NT):
        d = int(D_A[t] + D_B[t])
        if cur and ca + cb + d > CMAX:
            supers.append((cur, ca, cb, rowstart))
            rowstart += ca + cb
            cur, ca, cb = [], 0, 0
        cur.append(t)
        ca += int(D_A[t]); cb += int(D_B[t])
    supers.append((cur, ca, cb, rowstart))
    L = rowstart + ca + cb   # total gather rows (slots) per core

    # per tile row offsets within its super: A rect then B rect
    rowA0 = np.zeros(NT, np.int64)
    rowB0 = np.zeros(NT, np.int64)
    sup_of_tile = np.zeros(NT, np.int64)
    for si, (tl, sa, sb, rs) in enumerate(supers):
        accA, accB = 0, 0
        for t in tl:
            sup_of_tile[t] = si
            rowA0[t] = rs + accA
            rowB0[t] = rs + sa + accB
            accA += int(D_A[t]); accB += int(D_B[t])

    # dummy rows: local row 6250; dummyA on core 0, dummyB on core 6
    DUMA = 0 * SH + 6250
    DUMB = 6 * SH + 6250
    assert DUMA < ABOUND and DUMB >= OVLO

    # build per-core idx arrays
    pg_src = pg[src_all]
    pg_dst = pg[dst_all]
    e_core = pg_dst // SH
    e_l = pg_dst % SH
    e_t = e_l // 128
    e_p = e_l % 128

    # flex rank within (dst): cumcount over flex edges per dst
    nE = len(src_all)
    e_f = np.zeros(nE, np.int64)
    fsel = cat == 1
    fkey = pg_dst[fsel]
    forder = np.argsort(fkey, kind="stable")
    fj = _within_group_index(fkey[forder], None)
    tmp = np.empty(fsel.sum(), np.int64)
    tmp[forder] = fj
    e_f[fsel] = tmp
    flex_to_a = a_n - mA               # per node: # flex edges going to A
    e_half = np.where(cat == 0, 0,
                      np.where(cat == 2, 1,
                               (e_f >= flex_to_a[dst_all]).astype(np.int64)))

    # slot j within (dst, half): order by (pg_dst, half) stable
    ekey = pg_dst * 2 + e_half
    eorder = np.argsort(ekey, kind="stable")
    j_sorted = _within_group_index(ekey[eorder], None)
    e_j = np.empty(len(ekey), np.int64)
    e_j[eorder] = j_sorted

    rowX0 = np.where(e_half == 0, rowA0[e_t], rowB0[e_t])
    pos = (rowX0 + e_j) * 128 + e_p
    val = (pg_src - e_half * OVLO).astype(np.int16)

    # row dummy template (core independent)
    rowdum = np.empty(L, np.int16)
    for si, (tl, sa, sb, rs) in enumerate(supers):
        rowdum[rs:rs + sa] = DUMA
        rowdum[rs + sa:rs + sa + sb] = DUMB - OVLO
    base = np.repeat(rowdum, 128)

    idx_cores = []
    for c in range(NCORES):
        m = e_core == c
        flat = base.copy()
        flat[pos[m]] = val[m]
        # wrap per call segment: [n] -> [16, n/16]
        segs = []
        for (tl, sa, sb, rs) in supers:
            a = flat[rs * 128:(rs + sa) * 128]
            b = flat[(rs + sa) * 128:(rs + sa + sb) * 128]
            segs.append(a.reshape(-1, 16).T)
            segs.append(b.reshape(-1, 16).T)
        wrapped = np.concatenate(segs, axis=1)          # [16, L*8]
        idx_cores.append(np.tile(wrapped, (NCORES, 1)))  # [128, L*8]

    # xT per core: [512, 6272] bf16, columns = within-core l order
    xT_cores = []
    for c in range(NCORES):
        nodes_c = np.where(core == c)[0]
        lc = l[nodes_c]
        ordered = nodes_c[np.argsort(lc)]
        xt = np.zeros((C_IN, SH), np.float32)
        xt[:, :len(ordered)] = x[ordered].T
        xT_cores.append(xt.astype(ml_dtypes.bfloat16))

    # composed weight blocks
    v_s1 = (W1.reshape(C_IN, H1, F1) * as1[None]).sum(-1)   # [512, 8]
    v_d1 = (W1.reshape(C_IN, H1, F1) * ad1[None]).sum(-1)
    rhs1 = np.concatenate([W1, v_s1, v_d1], axis=1).astype(ml_dtypes.bfloat16)  # [512,80]
    v_s2 = W2 @ as2[0]                                       # [64]
    v_d2 = W2 @ ad2[0]
    rhs2 = np.concatenate([W2, v_s2[:, None], v_d2[:, None]], axis=1
                          ).astype(ml_dtypes.bfloat16)       # [64, 66]
    b1rep = np.tile(b1[None, :], (128, 1)).astype(np.float32)
    b2rep = np.tile(b2[None, :], (128, 1)).astype(np.float32)

    geom = dict(D_A=D_A, D_B=D_B, supers=supers, rowA0=rowA0, rowB0=rowB0,
                L=L)
    per_core = []
    for c in range(NCORES):
        per_core.append(dict(xT=xT_cores[c], idx=idx_cores[c], rhs1=rhs1,
                             rhs2=rhs2, b1rep=b1rep, b2rep=b2rep))
    return geom, per_core, core, l


def _within_group_index(sorted_keys, _):
    """For a sorted key array, return position of each element within its
    group of equal keys."""
    n = len(sorted_keys)
    starts = np.r_[True, sorted_keys[1:] != sorted_keys[:-1]]
    gidx = np.cumsum(starts) - 1
    start_pos = np.flatnonzero(starts)
    return np.arange(n) - start_pos[gidx]


# ================================ DEVICE BUILD =============================

def _build(geom):
    D_A, D_B = geom["D_A"], geom["D_B"]
    supers = geom["supers"]
    rowA0, rowB0 = geom["rowA0"], geom["rowB0"]
    L = geom["L"]

    nc = bacc.Bacc("TRN2", target_bir_lowering=False, debug=False,
                   num_devices=NCORES)

    xT_in = nc.dram_tensor("xT", [C_IN, SH], BF16, kind="ExternalInput")
    idx_in = nc.dram_tensor("idx", [128, L * 8], I16, kind="ExternalInput")
    rhs1_in = nc.dram_tensor("rhs1", [C_IN, 80], BF16, kind="ExternalInput")
    rhs2_in = nc.dram_tensor("rhs2", [64, 66], BF16, kind="ExternalInput")
    b1_in = nc.dram_tensor("b1rep", [128, 64], F32, kind="ExternalInput")
    b2_in = nc.dram_tensor("b2rep", [128, 64], F32, kind="ExternalInput")
    out_sl = nc.dram_tensor("out_slab", [SH, C_OUT], F32, kind="ExternalOutput")

    with tile.TileContext(nc) as tc:
        with (
            tc.tile_pool(name="const", bufs=1) as cp,
            tc.tile_pool(name="resident", bufs=1) as rp,
            tc.tile_pool(name="dram", bufs=1, space="DRAM") as dr,
        ):
            nc.gpsimd.load_library(mlp)

            ident = cp.tile([128, 128], BF16)
            make_identity(nc, ident[:])
            rhs1_sb = cp.tile([128, 4, 80], BF16)
            nc.sync.dma_start(
                rhs1_sb[:],
                bass.AP(rhs1_in[:].tensor, 0, [[80, 128], [128 * 80, 4], [1, 80]]))
            rhs2_sb = cp.tile([64, 66], BF16)
            nc.sync.dma_start(rhs2_sb[:], rhs2_in[:])
            b1_sb = cp.tile([128, 64], F32)
            nc.sync.dma_start(b1_sb[:], b1_in[:])
            b2_sb = cp.tile([128, 64], F32)
            nc.sync.dma_start(b2_sb[:], b2_in[:])

            idx_sb = rp.tile([128, L * 8], I16)
            nc.sync.dma_start(idx_sb[:], idx_in[:])

            astage = rp.tile([128, NT * 16], F32)     # [a_src1(8)|a_dst1(8)] per tile
            hTst = rp.tile([64, SH], BF16)            # my hpost^T slab stage
            a2st = rp.tile([128, NT, 2], F32)         # my [a_src2, a_dst2]

            # DRAM internals
            slab1 = dr.tile([SH, 64], F32)            # my table1 slab (256B rows)
            table1 = dr.tile([R, 64], F32)
            slabT = dr.tile([64, SH], BF16)
            tableT = dr.tile([NCORES * 64, SH], BF16)
            table2 = dr.tile([R, 64], F32)

            for rep in range(REPEAT):
                if rep == 0 or "A" in PHASES:
                    # ---------------- phase A: conv1 dense + slab build ----------
                    with (
                        tc.tile_pool(name="phA", bufs=1) as pa,
                        tc.tile_pool(name="psA", bufs=4, space="PSUM") as psa,
                    ):
                        xt_sb = []
                        for kc in range(4):
                            t_ = pa.tile([128, SH], BF16, name=f"xt{kc}")
                            nc.sync.dma_start(
                                t_[:],
                                bass.AP(xT_in[:].tensor, kc * 128 * SH,
                                        [[SH, 128], [1, SH]]))
                            xt_sb.append(t_)
                        h1st = pa.tile([128, NT * 64], BF16)
                        # pad rows of tile 48 (local rows 6250..6271): h=0, a_src=-big
                        # (memset full partitions first; tile-48 copies then cover
                        # only partitions 0:106)
                        nc.vector.memset(h1st[:, 48 * 64:49 * 64], 0.0)
                        nc.vector.memset(astage[:, 48 * 16:48 * 16 + 8], NEG_BIG)
                        nc.vector.memset(astage[:, 48 * 16 + 8:48 * 16 + 16], 0.0)
                        for t in range(NT):
                            ps = psa.tile([128, 80], F32, tag="psA", space="PSUM")
                            for kc in range(4):
                                nc.tensor.matmul(
                                    ps[:], lhsT=xt_sb[kc][:, t * 128:(t + 1) * 128],
                                    rhs=rhs1_sb[:, kc, :],
                                    start=(kc == 0), stop=(kc == 3))
                            pl = 106 if t == 48 else 128
                            nc.vector.tensor_copy(out=h1st[0:pl, t * 64:(t + 1) * 64],
                                                  in_=ps[0:pl, 0:64])
                            nc.vector.tensor_copy(out=astage[0:pl, t * 16:(t + 1) * 16],
                                                  in_=ps[0:pl, 64:80])

                        # slab1 writes: h (bf16 cols 0:64 of 128) + a (f32 cols 32:48)
                        sl_bf = slab1[:].bitcast(BF16)   # [SH, 128]
                        nc.sync.dma_start(
                            bass.AP(sl_bf.tensor, 0, [[128, 128], [128 * 128, NT], [1, 64]]),
                            bass.AP(h1st[:].tensor, h1st[:].offset,
                                    [h1st[:].ap[0], [64, NT], [1, 64]]))
                        nc.sync.dma_start(
                            bass.AP(slab1[:].tensor, 32, [[64, 128], [64 * 128, NT], [1, 16]]),
                            bass.AP(astage[:].tensor, astage[:].offset,
                                    [astage[:].ap[0], [16, NT], [1, 16]]))

                if rep == 0 or "B" in PHASES:
                    # ---------------- phase B: AllGather table1 ------------------
                    nc.gpsimd.collective_compute(
                        "AllGather", mybir.AluOpType.bypass,
                        replica_groups=[list(range(NCORES))],
                        ins=[slab1[:].opt()], outs=[table1[:].opt()])

                    if DEBUG_DUMPS and rep == 0:
                        d_t1 = nc.dram_tensor("d_table1", [R, 64], F32,
                                              kind="ExternalOutput")
                        nc.sync.dma_start(d_t1[:], table1[:])

                # ---------------- edge phase (shared for conv1/conv2) --------
                def edge_phase(table, heads, post_fn, nwide):
                    """nwide: matmul N (64 + heads). post_fn(si, tiles, pstage)"""
                    with (
                        tc.tile_pool(name="edge", bufs=2) as ep,
                        tc.tile_pool(name="post", bufs=2) as pp,
                        tc.tile_pool(name="psE", bufs=4, space="PSUM") as pse,
                    ):
                        for si, (tl, sa, sb_, rs) in enumerate(supers):
                            csup = sa + sb_
                            gbuf = ep.tile([128, csup, 64], F32, tag="gbuf",
                                           name=f"gbuf{si}")
                            ebuf = ep.tile([128, csup * heads], F32, tag="ebuf",
                                           name=f"ebuf{si}")
                            iofs = rs * 8
                            nc.gpsimd.dma_gather(
                                gbuf[:, 0:sa, :], table[0:ABOUND, :],
                                idx_sb[:, iofs:iofs + sa * 8],
                                sa * 128, sa * 128, 64, single_packet=False)
                            nc.gpsimd.dma_gather(
                                gbuf[:, sa:csup, :], table[R - ABOUND:R, :],
                                idx_sb[:, iofs + sa * 8:iofs + (sa + sb_) * 8],
                                sb_ * 128, sb_ * 128, 64, single_packet=False)

                            g_ap = gbuf[:].ap
                            g_off = gbuf[:].offset
                            g_t = gbuf[:].tensor
                            gb_bf = gbuf[:].bitcast(BF16)   # [128, csup, 128]
                            pstage = pp.tile([128, len(tl), 80], F32, tag="pst",
                                             name=f"pst{si}")

                            for ti, t in enumerate(tl):
                                ps = (pse.tile([128, nwide], F32, tag="psE",
                                               space="PSUM",
                                               name=f"psE{si}_{ti}")
                                      if ("M" in SUBPH or rep == 0) else None)
                                cols = []
                                for (r0, dd) in ((int(rowA0[t]) - rs, int(D_A[t])),
                                                 (int(rowB0[t]) - rs, int(D_B[t]))):
                                    cols += list(range(r0, r0 + dd))
                                    if "V" not in SUBPH and rep > 0:
                                        continue
                                    # e = a_src + a_dst  (f32 view cols 32:32+heads)
                                    asrc = bass.AP(g_t, g_off + r0 * 64 + 32,
                                                   [g_ap[0], [64, dd], [1, heads]])
                                    if heads == 8:
                                        adst = bass.AP(
                                            astage[:].tensor,
                                            astage[:].offset + t * 16 + 8,
                                            [astage[:].ap[0], [0, dd], [1, 8]])
                                    else:
                                        adst = bass.AP(
                                            a2st[:].tensor,
                                            a2st[:].offset + t * 2 + 1,
                                            [a2st[:].ap[0], [0, dd], [0, 1]])
                                    ebv = bass.AP(ebuf[:].tensor,
                                                  ebuf[:].offset + r0 * heads,
                                                  [ebuf[:].ap[0], [heads, dd],
                                                   [1, heads]])
                                    nc.vector.tensor_tensor(
                                        out=ebv, in0=asrc, in1=adst,
                                        op=mybir.AluOpType.add)
                                    # leaky relu: max(0.2*e, e)
                                    nc.vector.scalar_tensor_tensor(
                                        out=ebv, in0=ebv, scalar=NEG_SLOPE,
                                        in1=ebv, op0=mybir.AluOpType.mult,
                                        op1=mybir.AluOpType.max)
                                    # w = exp(e) -> bf16 into gbuf cols [64:64+heads]
                                    wv = bass.AP(gb_bf.tensor,
                                                 gb_bf.offset + r0 * 128 + 64,
                                                 [gb_bf.ap[0], [128, dd], [1, heads]])
                                    nc.scalar.activation(
                                        wv, ebv, mybir.ActivationFunctionType.Exp)
                                    # Mw in place on h region (bf16 [0:64])
                                    h4 = bass.AP(gb_bf.tensor,
                                                 gb_bf.offset + r0 * 128,
                                                 [gb_bf.ap[0], [128, dd],
                                                  [64 // heads, heads],
                                                  [1, 64 // heads]])
                                    w4 = bass.AP(gb_bf.tensor,
                                                 gb_bf.offset + r0 * 128 + 64,
                                                 [gb_bf.ap[0], [128, dd],
                                                  [1, heads], [0, 64 // heads]])
                                    nc.vector.tensor_tensor(
                                        out=h4, in0=h4, in1=w4,
                                        op=mybir.AluOpType.mult)
                                for k, cx in enumerate(cols) if ("M" in SUBPH or rep == 0) else []:
                                    nc.tensor.matmul(
                                        ps[:], lhsT=ident[:],
                                        rhs=bass.AP(gb_bf.tensor,
                                                    gb_bf.offset + cx * 128,
                                                    [gb_bf.ap[0], [1, nwide]]),
                                        start=(k == 0), stop=(k == len(cols) - 1))
                                if "M" in SUBPH or rep == 0:
                                    nc.vector.tensor_copy(
                                        out=pstage[:, ti, 0:nwide], in_=ps[:])
                            if "P" in SUBPH or rep == 0:
                                post_fn(si, tl, pstage)

                if rep == 0 or "C" in PHASES:
                    # ---------------- conv1 post: softmax div + bias + ELU + T ---
                    with (
                        tc.tile_pool(name="c1post", bufs=2) as c1p,
                        tc.tile_pool(name="c1postT", bufs=2, space="PSUM") as c1pt,
                    ):
                        def post1(si, tl, pstage):
                            g = len(tl)
                            p_t = pstage[:].tensor
                            p_o = pstage[:].offset
                            p_p = pstage[:].ap[0]
                            den = bass.AP(p_t, p_o + 64, [p_p, [80, g], [1, 8]])
                            rden = c1p.tile([128, g * 8], F32, tag="rden",
                                            name=f"rden{si}")
                            nc.vector.tensor_scalar_add(out=rden[:], in0=den,
                                                        scalar1=1e-16)
                            nc.vector.reciprocal(out=rden[:], in_=rden[:])
                            z = c1p.tile([128, g * 64], F32, tag="z", name=f"z{si}")
                            num = bass.AP(p_t, p_o, [p_p, [80, g], [8, 8], [1, 8]])
                            rd4 = bass.AP(rden[:].tensor, rden[:].offset,
                                          [rden[:].ap[0], [8, g], [1, 8], [0, 8]])
                            nc.vector.tensor_tensor(out=z[:], in0=num, in1=rd4,
                                                    op=mybir.AluOpType.mult)
                            b1b = bass.AP(b1_sb[:].tensor, b1_sb[:].offset,
                                          [b1_sb[:].ap[0], [0, g], [1, 64]])
                            nc.vector.tensor_tensor(out=z[:], in0=z[:], in1=b1b,
                                                    op=mybir.AluOpType.add)
                            # ELU(z) = max(z,0) + a*exp(min(z,0)) - a
                            m0 = c1p.tile([128, g * 64], F32, tag="m0", name=f"m0{si}")
                            nc.vector.tensor_scalar_min(out=m0[:], in0=z[:], scalar1=0.0)
                            nc.scalar.activation(m0[:], m0[:],
                                                 mybir.ActivationFunctionType.Exp)
                            nc.vector.tensor_scalar_max(out=z[:], in0=z[:], scalar1=0.0)
                            nc.vector.scalar_tensor_tensor(
                                out=z[:], in0=m0[:], scalar=ELU_ALPHA, in1=z[:],
                                op0=mybir.AluOpType.mult, op1=mybir.AluOpType.add)
                            nc.vector.tensor_scalar_add(out=z[:], in0=z[:],
                                                        scalar1=-ELU_ALPHA)
                            hp = c1p.tile([128, g * 64], BF16, tag="hp", name=f"hp{si}")
                            nc.vector.tensor_copy(out=hp[:], in_=z[:])
                            for ti, t in enumerate(tl):
                                pst = c1pt.tile([64, 128], BF16, tag="psT",
                                                space="PSUM", name=f"psT{si}_{ti}")
                                nc.tensor.transpose(
                                    out=pst[:], in_=hp[:, ti * 64:(ti + 1) * 64],
                                    identity=ident[:])
                                nc.vector.tensor_copy(
                                    out=hTst[:, t * 128:(t + 1) * 128], in_=pst[:])

                        edge_phase(table1, 8, post1, 72)

                    # a_dst2 (and a_src2) for my shard from hTst
                    with tc.tile_pool(name="a2", bufs=4, space="PSUM") as a2psp:
                        for t in range(NT):
                            a2ps = a2psp.tile([128, 2], F32, tag="a2ps", space="PSUM",
                                              name=f"a2ps{t}")
                            nc.tensor.matmul(a2ps[:],
                                             lhsT=hTst[:, t * 128:(t + 1) * 128],
                                             rhs=rhs2_sb[:, 64:66],
                                             start=True, stop=True)
                            nc.vector.tensor_copy(out=a2st[:, t, :], in_=a2ps[:])

                    if DEBUG_DUMPS and rep == 0:
                        d_ht = nc.dram_tensor("d_hTst", [64, SH], BF16,
                                              kind="ExternalOutput")
                        nc.sync.dma_start(d_ht[:], hTst[:])
                        d_a2 = nc.dram_tensor("d_a2st", [128, NT * 2], F32,
                                              kind="ExternalOutput")
                        nc.sync.dma_start(
                            d_a2[:],
                            bass.AP(a2st[:].tensor, a2st[:].offset,
                                    [a2st[:].ap[0], [2, NT], [1, 2]]))

                if rep == 0 or "D" in PHASES:
                    # ---------------- phase D: AllGather hpost^T -----------------
                    nc.sync.dma_start(slabT[:], hTst[:])
                    nc.gpsimd.collective_compute(
                        "AllGather", mybir.AluOpType.bypass,
                        replica_groups=[list(range(NCORES))],
                        ins=[slabT[:].opt()], outs=[tableT[:].opt()])

                if rep == 0 or "E" in PHASES:
                    # ---------------- phase E: conv2 table build (redundant) -----
                    with (
                        tc.tile_pool(name="phE", bufs=2) as pe,
                        tc.tile_pool(name="psEb", bufs=4, space="PSUM") as pseb,
                    ):
                        for b in range(NCORES):
                            ht = pe.tile([64, SH], BF16, tag="ht", name=f"ht{b}")
                            nc.sync.dma_start(ht[:], tableT[b * 64:(b + 1) * 64, :])
                            h2st = pe.tile([128, NT * 64], BF16, tag="h2st",
                                           name=f"h2st{b}")
                            a2stg = pe.tile([128, NT * 2], F32, tag="a2stg",
                                            name=f"a2stg{b}")
                            nc.vector.memset(a2stg[:, 48 * 2:48 * 2 + 1], NEG_BIG)
                            nc.vector.memset(a2stg[:, 48 * 2 + 1:48 * 2 + 2], 0.0)
                            nc.vector.memset(h2st[:, 48 * 64:49 * 64], 0.0)
                            for t in range(NT):
                                ps = pseb.tile([128, 66], F32, tag="psEb", space="PSUM",
                                               name=f"psEb{b}_{t}")
                                nc.tensor.matmul(ps[:],
                                                 lhsT=ht[:, t * 128:(t + 1) * 128],
                                                 rhs=rhs2_sb[:], start=True, stop=True)
                                pl = 106 if t == 48 else 128
                                nc.vector.tensor_copy(out=h2st[0:pl, t * 64:(t + 1) * 64],
                                                      in_=ps[0:pl, 0:64])
                                nc.vector.tensor_copy(out=a2stg[0:pl, t * 2:(t + 1) * 2],
                                                      in_=ps[0:pl, 64:66])
                            t2bf = table2[:].bitcast(BF16)
                            nc.sync.dma_start(
                                bass.AP(t2bf.tensor, b * SH * 128,
                                        [[128, 128], [128 * 128, NT], [1, 64]]),
                                bass.AP(h2st[:].tensor, h2st[:].offset,
                                        [h2st[:].ap[0], [64, NT], [1, 64]]))
                            nc.sync.dma_start(
                                bass.AP(table2[:].tensor, b * SH * 64 + 32,
                                        [[64, 128], [64 * 128, NT], [1, 2]]),
                                bass.AP(a2stg[:].tensor, a2stg[:].offset,
                                        [a2stg[:].ap[0], [2, NT], [1, 2]]))

                    if DEBUG_DUMPS and rep == 0:
                        d_t2 = nc.dram_tensor("d_table2", [R, 64], F32,
                                              kind="ExternalOutput")
                        nc.sync.dma_start(d_t2[:], table2[:])
                        d_as = nc.dram_tensor("d_astage", [128, NT * 16], F32,
                                              kind="ExternalOutput")
                        nc.sync.dma_start(d_as[:], astage[:])

                if rep == 0 or "F" in PHASES:
                    # ---------------- phase F: conv2 edge + output ---------------
                    with tc.tile_pool(name="c2post", bufs=2) as c2p:
                        outst = rp.tile([128, NT * 64], F32)

                        def post2(si, tl, pstage):
                            g = len(tl)
                            p_t = pstage[:].tensor
                            p_o = pstage[:].offset
                            p_p = pstage[:].ap[0]
                            den = bass.AP(p_t, p_o + 64, [p_p, [80, g], [1, 1]])
                            rden = c2p.tile([128, g], F32, tag="rd2", name=f"rd2{si}")
                            nc.vector.tensor_scalar_add(out=rden[:], in0=den,
                                                        scalar1=1e-16)
                            nc.vector.reciprocal(out=rden[:], in_=rden[:])
                            z = c2p.tile([128, g * 64], F32, tag="z2", name=f"z2{si}")
                            num = bass.AP(p_t, p_o, [p_p, [80, g], [1, 64]])
                            rd3 = bass.AP(rden[:].tensor, rden[:].offset,
                                          [rden[:].ap[0], [1, g], [0, 64]])
                            nc.vector.tensor_tensor(out=z[:], in0=num, in1=rd3,
                                                    op=mybir.AluOpType.mult)
                            b2b = bass.AP(b2_sb[:].tensor, b2_sb[:].offset,
                                          [b2_sb[:].ap[0], [0, g], [1, 64]])
                            for ti, t in enumerate(tl):
                                nc.vector.tensor_tensor(
                                    out=outst[:, t * 64:(t + 1) * 64],
                                    in0=z[:, ti * 64:(ti + 1) * 64],
                                    in1=b2_sb[:],
                                    op=mybir.AluOpType.add)

                        edge_phase(table2, 1, post2, 65)

                        nc.sync.dma_start(
                            bass.AP(out_sl[:].tensor, 0,
                                    [[64, 128], [64 * 128, NT], [1, 64]]),
                            bass.AP(outst[:].tensor, outst[:].offset,
                                    [outst[:].ap[0], [64, NT], [1, 64]]))

    nc.compile()
    return nc


# ================================ ENTRY ====================================

def kernel(x, edge_index, W1, att_src1, att_dst1, b1, W2, att_src2, att_dst2,
           b2):
    x = np.asarray(x, np.float32)
    edge_index = np.asarray(edge_index)
    W1 = np.asarray(W1, np.float32)
    as1 = np.asarray(att_src1, np.float32)
    ad1 = np.asarray(att_dst1, np.float32)
    b1 = np.asarray(b1, np.float32)
    W2 = np.asarray(W2, np.float32)
    as2 = np.asarray(att_src2, np.float32)
    ad2 = np.asarray(att_dst2, np.float32)
    b2 = np.asarray(b2, np.float32)

    geom, per_core, core, l = _host_prep(
        x, edge_index, W1, as1, ad1, b1, W2, as2, ad2, b2)
    nc = _build(geom)

    in_maps = [dict(xT=pc["xT"], idx=pc["idx"], rhs1=pc["rhs1"],
                    rhs2=pc["rhs2"], b1rep=pc["b1rep"], b2rep=pc["b2rep"])
               for pc in per_core]
    res = run_bass_kernel_spmd(nc, in_maps, core_ids=list(range(NCORES)),
                               trace=TRACE, **TRACE_KW)
    if TIME_RUNS:
        import time
        walls = []
        for i in range(TIME_RUNS):
            t0 = time.time()
            res = run_bass_kernel_spmd(nc, in_maps,
                                       core_ids=list(range(NCORES)))
            walls.append(time.time() - t0)
            print(f"run {i}: wall {walls[-1]:.3f}s  (REPEAT={REPEAT})")
        kernel.last_walls = walls
    kernel.last_results = res
    kernel.last_meta = (geom, core, l)
    if TRACE and res.exec_time_ns is not None:
        print(f"HW exec time: {res.exec_time_ns} ns")

    out = np.empty((N, C_OUT), np.float32)
    for c in range(NCORES):
        sl = res.results[c]["out_slab"]
        nodes_c = np.where(core == c)[0]
        out[nodes_c] = sl[l[nodes_c]]
    return out

